# revision 9
# baseline (speedup 1.0000x reference)
"""Trainium2 Bass kernel for: out = (x @ wsums.sum(0)) * (1.5 * 0.5).

x: [1024, 8192] f32, wsums: [32, 8192] f32 -> out: [1024, 1] f32.

Design (v4, PE-based): batch-parallel across the 8 cores (128 rows each).
The host pre-reduces wsums to wt = SCALE * wsums.sum(0), quantizes x to
int8 with one global scale s (wt absorbs s), and ships each core a
k-major transposed int8 image X8[p, t*128 + b] = xq[B0+b, t*128+p]
(1 MB/core) plus the bf16 w table wd[p, t] = wt[t*128+p] (16 KB).

Device per core:
  1. w DMA (HWDGE/sync).
  2. X stream: 5 SWDGE cast-DMAs (gpsimd) int8 HBM -> bf16 SBUF.  The
     SDMA engines are write-side bound (~175 ns per 4KB written), so the
     stream costs ~2.3MB/370GB/s; int8 still halves the HBM read side
     and the host->device input bytes.  Last piece is small (4 k-tiles)
     so the tail after the final sem is short.
  3. PE: 64 x (Ldweights [128,128] + Matmult N=1) accumulating
     psum[b, 0] += sum_p X8tile[p, b] * wd[p, g] over all 64 k-tiles.
     Matmults issue every ~27 ns (pipelined) - PE is never the
     bottleneck; the int8 quantization error (~1e-2 rel) sits well
     under the 2e-2 gate.
  4. DVE tensor_copy psum [128,1] fp32 -> bf16 acc, padded out-DMA
     [128, 256] (512B/partition avoids DRAM read-modify-write).
Host unshard: concatenate the 8 cores' 128-row slices (no partial-sum
rounding: each core computes its rows exactly).

Environment workarounds (this container's walrus build):
  - it encodes at most ONE semaphore wait per instruction ("Too many sync
    wait commands"), so compile_bir_kernel is wrapped with a BIR post-pass
    that moves excess waits onto preceding same-engine NoOp instructions;
  - Bass.__init__'s const-AP memsets and all-engine barrier are skipped
    (nothing here reads the const APs; the NRT start barrier already
    aligns engines), so Pool reaches the first SWDGE trigger earlier;
  - the stock TileContext exit (drain -> all-engine barrier -> sem clears
    -> barrier) is replaced with an overlapped exit: Sync drains on the
    out-DMA completion then increments a handoff semaphore; only GpSimd
    and Vector (whose walrus postamble slices S[105-155]/S[156-206]
    contain the live DMA/Tile semaphores) wait for it.
"""

import json

import ml_dtypes
import numpy as np

import concourse.bass as bass
import concourse.bass2jax as bass2jax
import concourse.bass_utils as bass_utils
import concourse.mybir as mybir
from concourse.tile import TileContext

SCALE = 1.5 * 0.5
B, K, G = 1024, 8192, 32
N_CORES = 8
P = 128
RB = B // N_CORES           # rows per core (128)
NT = K // P                 # 64 k-tiles per core
BF16 = mybir.dt.bfloat16
# k-tiles per stream piece (sum = NT).  Ramp up: the SWDGE descriptor ring
# holds roughly one 16-tile piece, so a large first piece stalls the second
# (measured 1.6us engine idle); small leading pieces keep the ring fed.
# Small last piece keeps the post-stream tail short.
PIECES = (4, 12, 16, 16, 12, 4)

# Set by test.py to profile; results stashed in LAST_RESULTS.
TRACE = False
TRACE_KWARGS = {}
LAST_RESULTS = None

_built = None

# ---------------------------------------------------------------------------
# Workaround: this container's walrus encodes at most 1 sync wait per
# instruction.  Split longer on_wait lists onto preceding same-engine NoOps.
MAX_WAITS = 1
_orig_compile_bir_kernel = bass_utils.compile_bir_kernel


def _split_waits_in_bir(bir: dict) -> int:
    counter = [0]

    def fix_blocks(blocks):
        for bb in blocks:
            out = []
            for ins in bb.get("instructions", []):
                si = ins.get("sync_info")
                ow = (si or {}).get("on_wait") or []
                if len(ow) > MAX_WAITS:
                    extra, keep = ow[:-MAX_WAITS], ow[-MAX_WAITS:]
                    for i in range(0, len(extra), MAX_WAITS):
                        counter[0] += 1
                        out.append({
                            "name": f"I-waitsplit-{counter[0]}",
                            "engine": ins["engine"],
                            "opcode": "NoOp",
                            "ins": [],
                            "outs": [],
                            "debug": ins.get("debug", 0),
                            "sync_info": {
                                "on_update": [],
                                "on_wait": extra[i : i + MAX_WAITS],
                            },
                        })
                    si["on_wait"] = keep
                out.append(ins)
            bb["instructions"] = out
            if bb.get("blocks"):
                fix_blocks(bb["blocks"])

    for fn in bir["functions"]:
        fix_blocks(fn["blocks"])
    return counter[0]


def _patched_compile_bir_kernel(bir_json, tmpdir, neff_name="file.neff"):
    if isinstance(bir_json, str):
        bir_json = bir_json.encode()
    bir = json.loads(bir_json)
    _split_waits_in_bir(bir)
    return _orig_compile_bir_kernel(json.dumps(bir).encode(), tmpdir, neff_name)


bass_utils.compile_bir_kernel = _patched_compile_bir_kernel
bass2jax.compile_bir_kernel = _patched_compile_bir_kernel


# ---------------------------------------------------------------------------
# Overlapped TileContext exit (see module docstring).
import concourse.tile as tile_mod
from concourse.tile import TileContext as _TC


def _overlap_drain_and_barrier(self, tick_clock, wait_clock):
    nc = self.nc
    drain_inst = nc.sync.drain()
    wait_clock.add_sem_waits(
        drain_inst.ins,
        tile_mod.ScopedClock({None: tick_clock.global_clock}),
    )
    done = nc.alloc_semaphore("tail_dma_done")
    # Must not sit in Tensor's or Scalar's postamble-clear slice (they are
    # released early and would zero it while GpSimd/Vector still wait).
    assert done.num >= 105, done.num
    drain_inst.then_inc(done, 1)
    nc.gpsimd.wait_ge(done, 1)
    nc.vector.wait_ge(done, 1)
    popped = nc._tile_sem_poison_stack.pop()
    assert popped is self._sem_poison


_TC._drain_and_barrier = _overlap_drain_and_barrier
# ---------------------------------------------------------------------------


def _build():
    # Skip Bass.__init__'s const-AP memsets + all-engine barrier: nothing
    # in this kernel reads the const APs, and the NRT start barrier already
    # aligns the engines, so Pool reaches the first SWDGE trigger sooner.
    _orig_aeb = bass.Bass.all_engine_barrier
    _orig_memset = bass.BassGpSimd.memset
    bass.Bass.all_engine_barrier = lambda self, **kw: None
    bass.BassGpSimd.memset = lambda self, *a, **kw: None
    try:
        nc = bass.Bass("TRN2")
    finally:
        bass.Bass.all_engine_barrier = _orig_aeb
        bass.BassGpSimd.memset = _orig_memset

    xd = nc.dram_tensor("x8", (P, K), mybir.dt.int8, kind="ExternalInput")
    wd = nc.dram_tensor("wt", (P, NT), BF16, kind="ExternalInput")
    # [32, 256] = 512B/partition on 32 partitions: avoids sub-512B DRAM
    # read-modify-write while using 4x fewer descriptors (and a faster
    # trigger) than a 128-partition write; host reads cols 0:4.
    out = nc.dram_tensor("out_acc", (32, 256), BF16, kind="ExternalOutput")

    with TileContext(nc) as tc, nc.allow_low_precision(
        reason="int8 x / bf16 w inputs; fp32 psum accumulation, 2e-2 gate"
    ):
        with (
            tc.tile_pool(name="const", bufs=1) as cpool,
            tc.tile_pool(name="xbuf", bufs=1) as xpool,
            tc.tile_pool(name="ps", bufs=1, space="PSUM") as ppool,
        ):
            w_sb = cpool.tile([P, NT], BF16)
            nc.sync.dma_start(out=w_sb, in_=bass.AP(wd, 0, [[NT, P], [1, NT]]))

            psum = ppool.tile([P, 1], mybir.dt.float32)
            acc4 = cpool.tile([32, 256], BF16)

            g = 0
            col = 0
            for pi, npc in enumerate(PIECES):
                xt = xpool.tile([P, npc * P], BF16, name=f"x{pi}")
                nc.gpsimd.dma_start(
                    out=xt, in_=bass.AP(xd, col, [[K, P], [1, npc * P]])
                )
                for t in range(npc):
                    nc.tensor.matmul(
                        psum[:, 0:1],
                        xt[:, t * P : (t + 1) * P],
                        w_sb[:, g : g + 1],
                        start=(g == 0),
                        stop=(g == NT - 1),
                    )
                    g += 1
                col += npc * P
            assert g == NT, g

            # Compact psum [128,1] fp32 onto 32 partitions as bf16:
            # acc4[i, k] = psum[32k + i].  Partition-base-shifted DVE
            # copies; each is tiny ([32,1]).
            for k in range(4):
                nc.vector.tensor_copy(
                    acc4[0:32, k : k + 1], psum[32 * k : 32 * (k + 1), 0:1]
                )
            nc.sync.dma_start(out=out.ap(), in_=acc4)
    return nc


def kernel(x: np.ndarray, wsums: np.ndarray) -> np.ndarray:
    global _built, LAST_RESULTS
    if _built is None:
        _built = _build()
    nc = _built

    x = np.asarray(x, dtype=np.float32)
    wsums = np.asarray(wsums, dtype=np.float32)

    # Quantize x to int8 with one global scale; fold scale (and SCALE) into w.
    s = float(np.abs(x).max()) / 127.0
    xq = np.rint(x * (1.0 / s)).astype(np.int8)        # [-127, 127]
    wt = (wsums.sum(axis=0, dtype=np.float32) * (SCALE * s)).astype(
        ml_dtypes.bfloat16
    )

    # Per-core k-major transpose: X8[c][p, t*128 + b] = xq[128c + b, t*128 + p]
    xt = np.ascontiguousarray(
        xq.reshape(N_CORES, RB, NT, P).transpose(0, 3, 2, 1)
    ).reshape(N_CORES, P, K)
    # wd[p, t] = wt[t*128 + p], replicated on every core.
    wd = np.ascontiguousarray(wt.reshape(NT, P).T)

    in_maps = [{"x8": xt[c], "wt": wd} for c in range(N_CORES)]

    res = bass_utils.run_bass_kernel_spmd(
        nc,
        in_maps,
        core_ids=list(range(N_CORES)),
        trace=TRACE,
        **TRACE_KWARGS,
    )
    LAST_RESULTS = res

    outv = np.empty((B,), dtype=np.float32)
    for c in range(N_CORES):
        a4 = res.results[c]["out_acc"][:, 0:4].astype(np.float32)  # [32, 4]
        outv[c * RB : (c + 1) * RB] = a4.T.reshape(RB)  # b_local = 32k + i
    return outv[:, None]


# revision 10
# speedup vs baseline: 1.6004x; 1.6004x over previous
"""Trainium2 Bass kernel for: out = (x @ wsums.sum(0)) * (1.5 * 0.5).

x: [1024, 8192] f32, wsums: [32, 8192] f32 -> out: [1024, 1] f32.

Design (v6, PE + w-sorted fp8/bf16 split): batch-parallel across the 8
cores (128 rows each).  The host pre-reduces wsums to
wt = SCALE * wsums.sum(0) and PERMUTES the contraction axis so the 7168
k's with the smallest |wt| come first; x columns for those k's are cast
to fp8e4m3 (their quantization error is weighted by the small |w| mass:
measured 7.3e-3 rel vs the 2e-2 gate), the remaining 1024 k's stay
bf16.  Each core receives a k-major transposed image of its 128 rows:
  xf8[p, t*128 + b] = x'[B0+b, t*128 + p]   (t in [0,56), fp8, 0.9 MB)
  xbf[p, u*128 + b] = x'[B0+b, (56+u)*128+p] (u in [0,8), bf16, 0.25 MB)
plus the permuted bf16 w table wd[p, t] = wt'[t*128+p] (16 KB).
1.15 MB/core streamed instead of 2 MB bf16 - the SDMA engines (which
are byte-bound on the SBUF write side at ~370 GB/s) finish in ~3.2 us.
All DMAs are HWDGE (sync queue): no SWDGE descriptor-ring stalls, no
cast, GpSimd never runs.

PE per core: 64 x (Ldweights [128,128] + Matmult N=1), x stationary
(fp8 tiles use FWL 4B weight reads), w moving, accumulating
psum[b, 0] += sum_p xtile[p, b] * wd[p, g] over all 64 k-tiles.
Matmults issue every ~27 ns pipelined - PE never binds.  Tail: one DVE
tensor_copy psum [128,1] fp32 -> bf16 into a [128, 256] padded tile
(512B/partition avoids DRAM read-modify-write) and one out-DMA.
Host unshard: concatenate the 8 cores' 128-row slices (each core's
rows are exact - no partial-sum rounding).

Environment workarounds (this container's walrus build):
  - it encodes at most ONE semaphore wait per instruction ("Too many sync
    wait commands"), so compile_bir_kernel is wrapped with a BIR post-pass
    that moves excess waits onto preceding same-engine NoOp instructions;
  - Bass.__init__'s const-AP memsets and all-engine barrier are skipped
    (nothing here reads the const APs; the NRT start barrier already
    aligns the engines);
  - the stock TileContext exit (drain -> all-engine barrier -> sem clears
    -> barrier) is replaced with an overlapped exit: Sync drains on the
    out-DMA completion then increments a handoff semaphore; only GpSimd
    and Vector (whose walrus postamble slices S[105-155]/S[156-206]
    contain the live DMA/Tile semaphores) wait for it.
"""

import json

import ml_dtypes
import numpy as np

import concourse.bass as bass
import concourse.bass2jax as bass2jax
import concourse.bass_utils as bass_utils
import concourse.mybir as mybir
from concourse.tile import TileContext

SCALE = 1.5 * 0.5
B, K, G = 1024, 8192, 32
N_CORES = 8
P = 128
RB = B // N_CORES           # rows per core (128)
NT = K // P                 # 64 k-tiles per core
NT8 = 56                    # leading k-tiles (smallest |w|) in fp8
NTB = NT - NT8              # trailing k-tiles in bf16
K8 = NT8 * P
KB16 = NTB * P
BF16 = mybir.dt.bfloat16
FP8 = mybir.dt.float8e4
# stream pieces: (dtype_region, ktile_start, ktile_count) over the
# permuted axis; fp8 region tiles [0,56), bf16 region tiles [56,64).
# Last piece small so the post-stream tail is short.
PIECES = (
    ("f8", 0, 16),
    ("f8", 16, 16),
    ("f8", 32, 16),
    ("bf", 56, 8),
    ("f8", 48, 8),
)

# Set by test.py to profile; results stashed in LAST_RESULTS.
TRACE = False
TRACE_KWARGS = {}
LAST_RESULTS = None

_built = None

# ---------------------------------------------------------------------------
# Workaround: this container's walrus encodes at most 1 sync wait per
# instruction.  Split longer on_wait lists onto preceding same-engine NoOps.
MAX_WAITS = 1
_orig_compile_bir_kernel = bass_utils.compile_bir_kernel


def _split_waits_in_bir(bir: dict) -> int:
    counter = [0]

    def fix_blocks(blocks):
        for bb in blocks:
            out = []
            for ins in bb.get("instructions", []):
                si = ins.get("sync_info")
                ow = (si or {}).get("on_wait") or []
                if len(ow) > MAX_WAITS:
                    extra, keep = ow[:-MAX_WAITS], ow[-MAX_WAITS:]
                    for i in range(0, len(extra), MAX_WAITS):
                        counter[0] += 1
                        out.append({
                            "name": f"I-waitsplit-{counter[0]}",
                            "engine": ins["engine"],
                            "opcode": "NoOp",
                            "ins": [],
                            "outs": [],
                            "debug": ins.get("debug", 0),
                            "sync_info": {
                                "on_update": [],
                                "on_wait": extra[i : i + MAX_WAITS],
                            },
                        })
                    si["on_wait"] = keep
                out.append(ins)
            bb["instructions"] = out
            if bb.get("blocks"):
                fix_blocks(bb["blocks"])

    for fn in bir["functions"]:
        fix_blocks(fn["blocks"])
    return counter[0]


def _patched_compile_bir_kernel(bir_json, tmpdir, neff_name="file.neff"):
    if isinstance(bir_json, str):
        bir_json = bir_json.encode()
    bir = json.loads(bir_json)
    _split_waits_in_bir(bir)
    return _orig_compile_bir_kernel(json.dumps(bir).encode(), tmpdir, neff_name)


bass_utils.compile_bir_kernel = _patched_compile_bir_kernel
bass2jax.compile_bir_kernel = _patched_compile_bir_kernel


# ---------------------------------------------------------------------------
# Overlapped TileContext exit (see module docstring).
import concourse.tile as tile_mod
from concourse.tile import TileContext as _TC


def _overlap_drain_and_barrier(self, tick_clock, wait_clock):
    nc = self.nc
    drain_inst = nc.sync.drain()
    wait_clock.add_sem_waits(
        drain_inst.ins,
        tile_mod.ScopedClock({None: tick_clock.global_clock}),
    )
    done = nc.alloc_semaphore("tail_dma_done")
    # Must not sit in Tensor's or Scalar's postamble-clear slice (they are
    # released early and would zero it while GpSimd/Vector still wait).
    assert done.num >= 105, done.num
    drain_inst.then_inc(done, 1)
    nc.gpsimd.wait_ge(done, 1)
    nc.vector.wait_ge(done, 1)
    popped = nc._tile_sem_poison_stack.pop()
    assert popped is self._sem_poison


_TC._drain_and_barrier = _overlap_drain_and_barrier
# ---------------------------------------------------------------------------


def _build():
    # Skip Bass.__init__'s const-AP memsets + all-engine barrier: nothing
    # in this kernel reads the const APs, and the NRT start barrier already
    # aligns the engines.
    _orig_aeb = bass.Bass.all_engine_barrier
    _orig_memset = bass.BassGpSimd.memset
    bass.Bass.all_engine_barrier = lambda self, **kw: None
    bass.BassGpSimd.memset = lambda self, *a, **kw: None
    try:
        nc = bass.Bass("TRN2")
    finally:
        bass.Bass.all_engine_barrier = _orig_aeb
        bass.BassGpSimd.memset = _orig_memset

    xf8 = nc.dram_tensor("xf8", (P, K8), FP8, kind="ExternalInput")
    xbf = nc.dram_tensor("xbf", (P, KB16), BF16, kind="ExternalInput")
    wd = nc.dram_tensor("wt", (P, NT), BF16, kind="ExternalInput")
    # Padded to 512B/partition: sub-512B DRAM writes read-modify-write in
    # the SDMA engines; host reads col 0.
    out = nc.dram_tensor("out_acc", (P, 256), BF16, kind="ExternalOutput")

    with TileContext(nc) as tc, nc.allow_low_precision(
        reason="fp8/bf16 x, bf16 w inputs; fp32 psum accumulation, 2e-2 gate"
    ):
        with (
            tc.tile_pool(name="const", bufs=1) as cpool,
            tc.tile_pool(name="xbuf", bufs=1) as xpool,
            tc.tile_pool(name="ps", bufs=1, space="PSUM") as ppool,
        ):
            w_sb = cpool.tile([P, NT], BF16)
            nc.sync.dma_start(out=w_sb, in_=bass.AP(wd, 0, [[NT, P], [1, NT]]))

            psum = ppool.tile([P, 1], mybir.dt.float32)
            acc = cpool.tile([P, 256], BF16)

            mm_idx = 0
            for pi, (kind, t0, npc) in enumerate(PIECES):
                if kind == "f8":
                    xt = xpool.tile([P, npc * P], FP8, name=f"x{pi}")
                    src = bass.AP(xf8, t0 * P, [[K8, P], [1, npc * P]])
                else:
                    xt = xpool.tile([P, npc * P], BF16, name=f"x{pi}")
                    src = bass.AP(xbf, (t0 - NT8) * P, [[KB16, P], [1, npc * P]])
                nc.sync.dma_start(out=xt, in_=src)
                for t in range(npc):
                    g = t0 + t
                    nc.tensor.matmul(
                        psum[:, 0:1],
                        xt[:, t * P : (t + 1) * P],
                        w_sb[:, g : g + 1],
                        start=(mm_idx == 0),
                        stop=(mm_idx == NT - 1),
                    )
                    mm_idx += 1
            assert mm_idx == NT, mm_idx

            nc.vector.tensor_copy(acc[:, 0:1], psum[:, 0:1])
            nc.sync.dma_start(out=out.ap(), in_=acc)
    return nc


def kernel(x: np.ndarray, wsums: np.ndarray) -> np.ndarray:
    global _built, LAST_RESULTS
    if _built is None:
        _built = _build()
    nc = _built

    x = np.asarray(x, dtype=np.float32)
    wsums = np.asarray(wsums, dtype=np.float32)

    wt = wsums.sum(axis=0, dtype=np.float32) * SCALE   # [K]
    # Permute k so the smallest-|wt| k's (whose x error matters least)
    # come first and get fp8; the largest-|wt| k's stay bf16.
    perm = np.argsort(np.abs(wt), kind="stable")
    xp = x[:, perm]
    wtp = wt[perm].astype(ml_dtypes.bfloat16)

    # Per-core k-major transpose: [c][p, t*128 + b] = xp[128c + b, t*128 + p]
    xt = xp.reshape(N_CORES, RB, NT, P).transpose(0, 3, 2, 1)  # [c, p, t, b]
    xt8 = np.ascontiguousarray(xt[:, :, :NT8]).reshape(N_CORES, P, K8)
    xtb = np.ascontiguousarray(xt[:, :, NT8:]).reshape(N_CORES, P, KB16)
    xt8 = xt8.astype(ml_dtypes.float8_e4m3fn)
    xtb = xtb.astype(ml_dtypes.bfloat16)
    # wd[p, t] = wtp[t*128 + p], replicated on every core.
    wd = np.ascontiguousarray(wtp.reshape(NT, P).T)

    in_maps = [
        {"xf8": xt8[c], "xbf": xtb[c], "wt": wd} for c in range(N_CORES)
    ]

    res = bass_utils.run_bass_kernel_spmd(
        nc,
        in_maps,
        core_ids=list(range(N_CORES)),
        trace=TRACE,
        **TRACE_KWARGS,
    )
    LAST_RESULTS = res

    outv = np.empty((B,), dtype=np.float32)
    for c in range(N_CORES):
        outv[c * RB : (c + 1) * RB] = res.results[c]["out_acc"][:, 0].astype(
            np.float32
        )
    return outv[:, None]
